# revision 10
# baseline (speedup 1.0000x reference)
"""Trainium2 Bass kernel for CrossAttention.

Reference computation (per batch item b):
    xt = x[b].reshape(C, N).T            # [N, C] tokens
    q = xt @ Wq.T + bq ; k = yt @ Wk.T + bk ; v = yt @ Wv.T + bv
    out = softmax(q @ k.T) @ v           # [N, C]
    return out.T.reshape(C, H, W)

Sharding: data-parallel over batch B=8 across the 8 NeuronCores (one batch
item per core). Each core holds the full 256x256 projection weights.

Device-side scheme (per core):
  - Softmax drops per-query-row constants, so
        scores = q k^T = X^T (Wq^T Wk) Y + 1 (Wk^T bq)^T Y  (+ row consts).
    M = Wq^T Wk and u = Wk^T bq are computed ONCE outside the repeat loop;
    in-loop there is a single projection Xt = M^T X + u (per-partition bias
    on the Pool engine) and the raw y tile is the scores stationary -- the
    whole K projection disappears from the loop.
  - scoresT[kv, q] = Y_chunk^T @ Xt accumulated over the two C/2 halves into
    512-wide PSUM tiles; Act engine exps them straight into bf16 probsT
    tiles (the PV matmul's lhsT -- no transpose needed).
  - V = Y^T Wv in [n, C] layout (Pool-engine copy to bf16) with two ones
    columns appended -> the PV matmul emits the softmax denominator free.
  - PV: out[q, 0:256] / out[q, 256] via DVE reciprocal + scale (bf16), then
    a bf16 PE transpose per 128-column block makes the output [c, q]; bias
    bv is added by DVE during the transpose drain (per-partition there).
  - Engine budget per iteration (cost model): PE ~79us, Act ~51us (exp
    only), DVE ~18us, Pool ~18us; the out DMA (2.4MB) overlaps the next
    iteration's projections.
"""

import numpy as np

import concourse.bass as bass
import concourse.mybir as mybir
import concourse.tile as tile
from concourse import bacc
from concourse.bass_utils import run_bass_kernel_spmd
from concourse.masks import make_identity

B, C, H, W = 8, 256, 48, 48
NTOK = H * W  # 2304
N_CORES = 8

DT = mybir.dt.float32
DTR = mybir.dt.float32r
BF = mybir.dt.bfloat16
FP = mybir.ActivationFunctionType


def build_program(ntok=NTOK, repeat=1, qw=512, stages=3):
    """Build the per-core SPMD Bass program."""
    nkv = ntok // 128          # kv chunks of 128 tokens
    n_half = 2                 # C=256 -> two 128-partition halves
    # query blocks: qw-wide except a ragged tail
    q_blocks = []
    q0 = 0
    while q0 < ntok:
        q_blocks.append((q0, min(qw, ntok - q0)))
        q0 += qw

    nc = bacc.Bacc("TRN2", target_bir_lowering=False, debug=False,
                   num_devices=N_CORES)

    x_d = nc.dram_tensor("x", [C, ntok], DTR, kind="ExternalInput").ap()
    y_d = nc.dram_tensor("y", [C, ntok], DTR, kind="ExternalInput").ap()
    wq_d = nc.dram_tensor("wq", [C, C], DTR, kind="ExternalInput").ap()
    wk_d = nc.dram_tensor("wk", [C, C], DTR, kind="ExternalInput").ap()
    wvt_d = nc.dram_tensor("wvt", [C, C], DTR, kind="ExternalInput").ap()
    u_d = nc.dram_tensor("u", [C], DT, kind="ExternalInput").ap()
    bv_d = nc.dram_tensor("bv", [C], DT, kind="ExternalInput").ap()
    out_d = nc.dram_tensor("out", [C, ntok], DT, kind="ExternalOutput").ap()

    with tile.TileContext(nc) as tc:
        with (
            tc.tile_pool(name="const", bufs=1) as constp,
            tc.tile_pool(name="xy", bufs=1) as xyp,
            tc.tile_pool(name="qk", bufs=1) as qkp,
            tc.tile_pool(name="vw", bufs=1) as vwp,
            tc.tile_pool(name="probs", bufs=3) as probsp,
            tc.tile_pool(name="epi", bufs=5) as epip,
            tc.tile_pool(name="outp", bufs=1) as outp,
            tc.tile_pool(name="ps_s", bufs=4, space="PSUM") as ps_sp,
            tc.tile_pool(name="ps_pv", bufs=2, space="PSUM") as ps_pvp,
            tc.tile_pool(name="ps_tr", bufs=2, space="PSUM") as ps_trp,
        ):
            ident = constp.tile([128, 128], BF)
            make_identity(nc, ident[:])

            x_t = xyp.tile([128, n_half, ntok], DTR, tag="x")
            y_t = xyp.tile([128, n_half, ntok], DTR, tag="y")
            xr = x_d.rearrange("(kh p) n -> p kh n", p=128)
            yr = y_d.rearrange("(kh p) n -> p kh n", p=128)
            nchunk = ntok // 4
            for ci in range(4):
                n0 = ci * nchunk
                nc.sync.dma_start(x_t[:, :, n0:n0 + nchunk], xr[:, :, n0:n0 + nchunk])
                nc.sync.dma_start(y_t[:, :, n0:n0 + nchunk], yr[:, :, n0:n0 + nchunk])

            wqr_t = constp.tile([128, n_half, C], DTR, tag="wqr")
            wkr_t = constp.tile([128, n_half, C], DTR, tag="wkr")
            wv_t = constp.tile([128, n_half, C], DTR, tag="wv")
            nc.sync.dma_start(wqr_t[:], wq_d.rearrange("(kh p) n -> p kh n", p=128))
            nc.sync.dma_start(wkr_t[:], wk_d.rearrange("(kh p) n -> p kh n", p=128))
            nc.sync.dma_start(wv_t[:], wvt_d.rearrange("(kh p) n -> p kh n", p=128))
            u_t = constp.tile([128, n_half], DT, tag="u")
            bv_t = constp.tile([128, n_half], DT, tag="bv")
            nc.sync.dma_start(u_t[:], u_d.rearrange("(kh p) -> p kh", p=128))
            nc.sync.dma_start(bv_t[:], bv_d.rearrange("(kh p) -> p kh", p=128))

            # ---- once: M = Wq^T Wk  ([c_x, c_y], contraction over c_out) ----
            m_t = constp.tile([128, n_half, C], DTR, tag="m")
            for khx in range(n_half):
                ps = ps_sp.tile([128, qw], DT, tag="ps_s")
                for kho in range(n_half):
                    nc.tensor.matmul(
                        ps[:, 0:C],
                        wqr_t[:, kho, khx * 128:(khx + 1) * 128],
                        wkr_t[:, kho, :],
                        start=(kho == 0), stop=(kho == n_half - 1),
                    )
                nc.scalar.activation(m_t[:, khx, :], ps[:, 0:C], FP.Copy)
            # V tile (ones columns set once; the loop only rewrites [:, :, 0:C])
            v_t = vwp.tile([128, nkv, C + 2], BF, tag="v")
            nc.vector.memset(v_t[:, :, C:C + 2], 1.0)

            import contextlib
            loop_cm = (tc.For_i(0, repeat, 1) if repeat > 1
                       else contextlib.nullcontext())
            with loop_cm:
                # ---- projection: Xt[c, n] = M^T X + u ----
                xt_t = qkp.tile([128, n_half, ntok], DTR, tag="xt")
                for cc in range(n_half):
                    for (n0, nw) in q_blocks:
                        ps = ps_sp.tile([128, qw], DT, tag="ps_s")
                        for kh in range(n_half):
                            nc.tensor.matmul(
                                ps[:, 0:nw],
                                m_t[:, kh, cc * 128:(cc + 1) * 128],
                                x_t[:, kh, n0:n0 + nw],
                                start=(kh == 0), stop=(kh == n_half - 1),
                            )
                        nc.vector.tensor_scalar_add(
                            xt_t[:, cc, n0:n0 + nw], ps[:, 0:nw],
                            u_t[:, cc:cc + 1])

                # ---- projection V in [n, c] layout (bf16) ----
                for j in range(nkv):
                    ps = ps_sp.tile([128, qw], DT, tag="ps_s")
                    for kh in range(n_half):
                        nc.tensor.matmul(
                            ps[:, 0:C],
                            y_t[:, kh, j * 128:(j + 1) * 128],
                            wv_t[:, kh, :],
                            start=(kh == 0), stop=(kh == n_half - 1),
                        )
                    nc.scalar.copy(v_t[:, j, 0:C], ps[:, 0:C])

                # ---- attention ----
                out_all = outp.tile([128, n_half, ntok], DT, tag="oall")

                def emit_epilogue(items):
                    for (o_sb, nq0) in items:
                        for cc in range(n_half):
                            pt = ps_trp.tile([128, 128], BF, tag="pt")
                            nc.tensor.transpose(
                                pt[:], o_sb[:, cc * 128:(cc + 1) * 128],
                                ident[:])
                            nc.vector.tensor_scalar_add(
                                out_all[:, cc, nq0:nq0 + 128], pt[:],
                                bv_t[:, cc:cc + 1])

                deferred = []
                for (q0, qwi) in q_blocks:
                    pbt = probsp.tile([128, nkv, qw], BF, tag="pbt")
                    for j in range(nkv):
                        ps = ps_sp.tile([128, qw], DT, tag="ps_s")
                        for kh in range(n_half):
                            nc.tensor.matmul(
                                ps[:, 0:qwi],
                                y_t[:, kh, j * 128:(j + 1) * 128],
                                xt_t[:, kh, q0:q0 + qwi],
                                start=(kh == 0), stop=(kh == n_half - 1),
                            )
                        nc.scalar.activation(pbt[:, j, 0:qwi], ps[:, 0:qwi],
                                             FP.Exp)

                    emit_epilogue(deferred)
                    deferred = []
                    for qq in range(qwi // 128):
                        po = ps_pvp.tile([128, C + 2], DT, tag="po")
                        for j in range(nkv):
                            nc.tensor.matmul(
                                po[:],
                                pbt[:, j, qq * 128:(qq + 1) * 128],
                                v_t[:, j, :],
                                start=(j == 0), stop=(j == nkv - 1),
                            )
                        r_t = epip.tile([128, 1], DT, tag="r")
                        nc.vector.reciprocal_approx_fast(r_t[:], po[:, C:C + 1])
                        o_sb = epip.tile([128, C], BF, tag="osb")
                        nc.vector.tensor_scalar_mul(o_sb[:], po[:, 0:C], r_t[:])
                        deferred.append((o_sb, q0 + qq * 128))
                emit_epilogue(deferred)
                nc.sync.dma_start(
                    out_d.rearrange("(cc p) n -> p cc n", p=128),
                    out_all[:])

    nc.compile()
    return nc


_CACHE = {}


def _get_program(ntok=NTOK):
    key = ntok
    if key not in _CACHE:
        _CACHE[key] = build_program(ntok=ntok)
    return _CACHE[key]


def kernel(x, y, Wq, bq, Wk, bk, Wv, bv):
    x = np.ascontiguousarray(np.asarray(x, dtype=np.float32))
    y = np.ascontiguousarray(np.asarray(y, dtype=np.float32))
    Wq = np.ascontiguousarray(np.asarray(Wq, dtype=np.float32))
    Wk = np.ascontiguousarray(np.asarray(Wk, dtype=np.float32))
    Wv = np.asarray(Wv, dtype=np.float32)
    bq = np.ascontiguousarray(np.asarray(bq, dtype=np.float32))
    bv = np.ascontiguousarray(np.asarray(bv, dtype=np.float32))

    b, c, h, w = x.shape
    ntok = h * w
    wvt = np.ascontiguousarray(Wv.T)
    u = np.ascontiguousarray((Wk.T @ bq).astype(np.float32))

    nc = _get_program(ntok)
    in_maps = []
    for i in range(N_CORES):
        in_maps.append({
            "x": x[i].reshape(c, ntok),
            "y": y[i].reshape(c, ntok),
            "wq": Wq, "wk": Wk, "wvt": wvt,
            "u": u, "bv": bv,
        })
    res = run_bass_kernel_spmd(nc, in_maps, list(range(N_CORES)))
    out = np.empty((b, c, h, w), dtype=np.float32)
    for i in range(N_CORES):
        out[i] = res.results[i]["out"].reshape(c, h, w)
    return out
